# revision 21
# baseline (speedup 1.0000x reference)
"""BitNet ternary linear layer on 8 Trainium2 NeuronCores.

y = x @ (W * s)^T with x (32, 4096) f32, W (11008, 4096) ternary {-1,0,+1}.

Strategy (memory-bound — the kernel is a W-stream at HBM line rate):
  - Tensor-parallel: shard W rows (out_features) across 8 cores, 1376 each;
    x replicated; per-core [32, 1376] outputs concatenated on the host.
  - Host-side prep (free): fold s into x, transpose to PE layouts, store W
    as fp8 E4M3 (ternary is EXACT in fp8). x is split into NSPLIT=2 fp8
    planes stacked along the stationary M dim (~2^-8 x precision; measured
    rel err ~7e-4 vs the 2e-2 gate), so x DMA is only 262 KB.
  - fp8 DoubleRow matmuls: K=256 per pass, 16 passes accumulate into
    per-chunk PSUM tiles (one tile per 512-col output chunk so each chunk's
    PSUM->SBUF copy + out-DMA depends only on its own accumulation group
    and overlaps the remaining matmuls).
  - W DRAM layout is k-major per partition; stripes ring on both HWDGE
    queues (Sync + Scalar), small-first for bandwidth ramp, small-last so
    the final matmuls are gated on as few bytes as possible.
  - Last TAIL_PASSES passes run chunk-major: chunk c's accumulation closes
    early, staging/DMA for c overlaps matmuls of c+1.
  - Outputs leave as fp16 raw planes (cast during the PSUM->SBUF copy);
    the scaled plane-sum runs on the host in f32.
  - Warmup/filler matmuls keep the PE busy so the HAM clock gate reaches
    K=8/8 (2.4 GHz) early instead of idling back to 1.2 GHz.
"""

import numpy as np
import ml_dtypes

N_CORES = 8
B, I, O = 32, 4096, 11008
OC = O // N_CORES        # 1376
NP = I // 256            # 16 DoubleRow passes (K=256 each)
NSPLIT = 2               # fp8 planes of x
ALPHA = 16.0             # residual plane q scaled by ALPHA**q
M = NSPLIT * B           # 64 stationary columns
# W DMA stripes in DoubleRow passes: small-first (bandwidth ramp), big
# middle, small-last (short critical path for the final matmuls).
STRIPE_PASSES = [1, 1, 1, 1, 2, 2, 2, 2, 2, 1, 1]
STRIPE_OFF = np.cumsum([0] + STRIPE_PASSES).tolist()
OCHUNKS = [(0, 512), (512, 512), (1024, 352)]
WARMUP_MMS = 7
TAIL_PASSES = 2          # chunk-major over the last N passes

_BUILT = None


def _build():
    import concourse.bacc as bacc
    import concourse.mybir as mybir
    from concourse.tile import TileContext

    f8 = mybir.dt.float8e4
    f16 = mybir.dt.float16
    nc = bacc.Bacc("TRN2", target_bir_lowering=False, debug=False)
    xt = nc.dram_tensor("xt", (128, NP * 2 * M), f8, kind="ExternalInput")
    wt = nc.dram_tensor("wt", (128, NP * 2 * OC), f8, kind="ExternalInput")
    # raw per-plane partials in fp16; the scaled plane-sum happens on host
    yp = nc.dram_tensor("yp", (M, OC), f16, kind="ExternalOutput")

    with TileContext(nc) as tc:
        with (
            tc.tile_pool(name="xp", bufs=1) as xp,
            tc.tile_pool(name="wp", bufs=1) as wp,
            tc.tile_pool(name="pp", bufs=1, space="PSUM") as pp,
            tc.tile_pool(name="op", bufs=1) as op,
        ):
            # PE warmup: garbage matmuls on a memset tile (no DMA dependency)
            # into a scratch PSUM bank, warming HAM while x and W stripe 0
            # load.
            wsrc = xp.tile([128, 512], f8, name="wsrc")
            nc.gpsimd.memset(wsrc[:, :], 0.0)
            scratch = pp.tile([128, 512], mybir.dt.float32, name="scratch")
            for wu in range(WARMUP_MMS):
                nc.tensor.matmul(
                    scratch[:, :], wsrc[:, 0:128], wsrc[:, 0:512],
                    start=True, stop=True,
                )

            # x rides the gpsimd SWDGE ring: a third DMA path, so both
            # HWDGE rings stream W from their first byte.
            xs = xp.tile([128, NP * 2 * M], f8)
            nc.gpsimd.dma_start(xs[:, :], xt[:, :])

            # stripe 0 rings on Scalar's HWDGE queue while the x DMA rings
            # on Sync's — both land together, real matmuls start early.
            stripes = []
            for s, np_s in enumerate(STRIPE_PASSES):
                w = wp.tile([128, np_s * 2 * OC], f8, name=f"w{s}", tag=f"w{s}")
                o0 = STRIPE_OFF[s] * 2 * OC
                eng = nc.scalar if s % 2 == 0 else nc.sync
                eng.dma_start(w[:, :], wt[:, o0 : o0 + np_s * 2 * OC])
                stripes.append(w)

            # One PSUM tile per output chunk: each matmul writes one bank,
            # and chunk c's staging depends only on chunk c's group.
            ps = [
                pp.tile([M, n], mybir.dt.float32, name=f"ps{c}")
                for c, (o0, n) in enumerate(OCHUNKS)
            ]
            import bisect

            def mm(j, c):
                o0, n = OCHUNKS[c]
                s = bisect.bisect_right(STRIPE_OFF, j) - 1
                jj = j - STRIPE_OFF[s]
                w4 = stripes[s][:, :].rearrange(
                    "p (jj i o) -> p jj i o", jj=STRIPE_PASSES[s], i=2, o=OC
                )
                nc.tensor.matmul(
                    ps[c][:, :],
                    x4[:, j],
                    w4[:, jj, :, o0 : o0 + n],
                    start=(j == 0),
                    stop=(j == NP - 1),
                    perf_mode=mybir.MatmulPerfMode.DoubleRow,
                )

            x4 = xs[:, :].rearrange("p (j i m) -> p j i m", j=NP, i=2, m=M)
            # j-major through pass 14: pass-14 matmuls run as soon as its
            # stripe lands (mid-stream) so only the 3 closing pass-15
            # matmuls remain after the final stripe's semaphore.
            for j in range(NP - 1):
                for c in range(len(OCHUNKS)):
                    mm(j, c)
                # filler matmuls: early j-groups are DMA-gated with PE-idle
                # gaps that keep resetting the HAM activity window; fillers
                # keep the PE busy until it reaches K=8/8 (2.4 GHz).
                if j < 4:
                    for f in range(2):
                        nc.tensor.matmul(
                            scratch[:, :], wsrc[:, 0:128], wsrc[:, 0:512],
                            start=True, stop=True,
                        )
            # close each chunk, then drain: c0 on vector and c1 on scalar
            # run fully parallel; c2 (smallest, closes last) splits its
            # CAST across both engines the moment they free up.
            dma_eng = (nc.sync, nc.scalar, nc.sync)
            sbs = []
            for c, (o0, n) in enumerate(OCHUNKS):
                mm(NP - 1, c)
                sb = op.tile([M, n], f16, name=f"sb{c}", tag=f"sb{c}")
                sbs.append(sb)
                if c == 0:
                    nc.vector.tensor_copy(sb[:, :], ps[c][:, :])
                elif c == 1:
                    nc.scalar.copy(sb[:, :], ps[c][:, :])
                else:
                    h = n // 2
                    nc.vector.tensor_copy(sb[:, 0:h], ps[c][:, 0:h])
                    nc.scalar.copy(sb[:, h:n], ps[c][:, h:n])
                dma_eng[c].dma_start(yp[:, o0 : o0 + n], sb[:, :])

    nc.finalize()
    return nc


def _get_nc():
    global _BUILT
    if _BUILT is None:
        _BUILT = _build()
    return _BUILT


def _fp8_split(v, nsplit):
    """Split v into fp8 planes: v ~= sum_q planes[q] / ALPHA**q."""
    planes = []
    rem = v.astype(np.float32)
    for q in range(nsplit):
        p = (rem * np.float32(ALPHA**q)).astype(ml_dtypes.float8_e4m3fn)
        planes.append(p)
        rem = rem - p.astype(np.float32) / np.float32(ALPHA**q)
    return planes


def _prep_inputs(x, weight, scale_factor):
    x = np.asarray(x, dtype=np.float32)
    weight = np.asarray(weight, dtype=np.float32)
    s = np.float32(np.asarray(scale_factor))

    xsT = (x * s).T.astype(np.float32)                  # [I, B]
    planes = _fp8_split(xsT, NSPLIT)
    stacked = np.concatenate(planes, axis=1)            # [I, M]
    # [I, M] with I = (j, i, p): k = 256j + 128i + p  ->  xt[p, j, i, m]
    xt = np.ascontiguousarray(
        stacked.reshape(NP, 2, 128, M).transpose(2, 0, 1, 3).reshape(128, NP * 2 * M)
    )

    in_maps = []
    for c in range(N_CORES):
        wc = weight[c * OC : (c + 1) * OC, :]           # [OC, I]
        wq = wc.T.astype(ml_dtypes.float8_e4m3fn)       # [I, OC], exact
        wtc = np.ascontiguousarray(
            wq.reshape(NP, 2, 128, OC).transpose(2, 0, 1, 3).reshape(128, NP * 2 * OC)
        )
        in_maps.append({"xt": xt, "wt": wtc})
    return in_maps


def _run(in_maps, trace=False, tmpdir=None):
    from concourse.bass_utils import run_bass_kernel_spmd

    return run_bass_kernel_spmd(
        _get_nc(), in_maps, core_ids=list(range(N_CORES)), trace=trace, tmpdir=tmpdir
    )


def _combine(yp):
    acc = yp[0:B].astype(np.float32).copy()
    for q in range(1, NSPLIT):
        acc += yp[q * B : (q + 1) * B].astype(np.float32) * np.float32(
            1.0 / ALPHA**q
        )
    return acc


def kernel(x, weight, scale_factor):
    in_maps = _prep_inputs(x, weight, scale_factor)
    try:
        res = _run(in_maps)
    except Exception:
        # transient runtime/device hiccups happen; one retry is cheap and
        # the output is still checked downstream
        res = _run(in_maps)
    return np.concatenate(
        [_combine(res.results[c]["yp"]) for c in range(N_CORES)], axis=1
    )



# revision 22
# speedup vs baseline: 1.0127x; 1.0127x over previous
"""BitNet ternary linear layer on 8 Trainium2 NeuronCores.

y = x @ (W * s)^T with x (32, 4096) f32, W (11008, 4096) ternary {-1,0,+1}.

Strategy (memory-bound — the kernel is a W-stream at HBM line rate):
  - Tensor-parallel: shard W rows (out_features) across 8 cores, 1376 each;
    x replicated; per-core [32, 1376] outputs concatenated on the host.
  - Host-side prep (free): fold s into x, transpose to PE layouts, store W
    as fp8 E4M3 (ternary is EXACT in fp8). x is split into NSPLIT=2 fp8
    planes stacked along the stationary M dim (~2^-8 x precision; measured
    rel err ~7e-4 vs the 2e-2 gate), so x DMA is only 262 KB.
  - fp8 DoubleRow matmuls: K=256 per pass, 16 passes accumulate into
    per-chunk PSUM tiles (one tile per 512-col output chunk so each chunk's
    PSUM->SBUF copy + out-DMA depends only on its own accumulation group
    and overlaps the remaining matmuls).
  - W DRAM layout is k-major per partition; stripes ring on both HWDGE
    queues (Sync + Scalar), small-first for bandwidth ramp, small-last so
    the final matmuls are gated on as few bytes as possible.
  - Last TAIL_PASSES passes run chunk-major: chunk c's accumulation closes
    early, staging/DMA for c overlaps matmuls of c+1.
  - Outputs leave as fp16 raw planes (cast during the PSUM->SBUF copy);
    the scaled plane-sum runs on the host in f32.
  - Warmup/filler matmuls keep the PE busy so the HAM clock gate reaches
    K=8/8 (2.4 GHz) early instead of idling back to 1.2 GHz.
"""

import numpy as np
import ml_dtypes

N_CORES = 8
B, I, O = 32, 4096, 11008
OC = O // N_CORES        # 1376
NP = I // 256            # 16 DoubleRow passes (K=256 each)
NSPLIT = 2               # fp8 planes of x
ALPHA = 16.0             # residual plane q scaled by ALPHA**q
M = NSPLIT * B           # 64 stationary columns
# W DMA stripes in DoubleRow passes: small-first (bandwidth ramp), big
# middle, small-last (short critical path for the final matmuls).
STRIPE_PASSES = [1, 1, 1, 1, 2, 2, 2, 2, 2, 1, 1]
STRIPE_OFF = np.cumsum([0] + STRIPE_PASSES).tolist()
OCHUNKS = [(0, 512), (512, 512), (1024, 352)]
WARMUP_MMS = 7
TAIL_PASSES = 2          # chunk-major over the last N passes

_BUILT = None


def _build():
    import concourse.bacc as bacc
    import concourse.mybir as mybir
    from concourse.tile import TileContext

    f8 = mybir.dt.float8e4
    f16 = mybir.dt.float16
    nc = bacc.Bacc("TRN2", target_bir_lowering=False, debug=False)
    xt = nc.dram_tensor("xt", (128, NP * 2 * M), f8, kind="ExternalInput")
    wt = nc.dram_tensor("wt", (128, NP * 2 * OC), f8, kind="ExternalInput")
    # raw per-plane partials in fp16; the scaled plane-sum happens on host
    yp = nc.dram_tensor("yp", (M, OC), f16, kind="ExternalOutput")

    with TileContext(nc) as tc:
        with (
            tc.tile_pool(name="xp", bufs=1) as xp,
            tc.tile_pool(name="wp", bufs=1) as wp,
            tc.tile_pool(name="pp", bufs=1, space="PSUM") as pp,
            tc.tile_pool(name="op", bufs=1) as op,
        ):
            # PE warmup: garbage matmuls on a memset tile (no DMA dependency)
            # into a scratch PSUM bank, warming HAM while x and W stripe 0
            # load.
            wsrc = xp.tile([128, 512], f8, name="wsrc")
            nc.gpsimd.memset(wsrc[:, :], 0.0)
            scratch = pp.tile([128, 512], mybir.dt.float32, name="scratch")
            for wu in range(WARMUP_MMS):
                nc.tensor.matmul(
                    scratch[:, :], wsrc[:, 0:128], wsrc[:, 0:512],
                    start=True, stop=True,
                )

            xs = xp.tile([128, NP * 2 * M], f8)
            nc.sync.dma_start(xs[:, :], xt[:, :])

            # stripe 0 rings on Scalar's HWDGE queue while the x DMA rings
            # on Sync's — both land together, real matmuls start early.
            stripes = []
            for s, np_s in enumerate(STRIPE_PASSES):
                w = wp.tile([128, np_s * 2 * OC], f8, name=f"w{s}", tag=f"w{s}")
                o0 = STRIPE_OFF[s] * 2 * OC
                eng = nc.scalar if s % 2 == 0 else nc.sync
                eng.dma_start(w[:, :], wt[:, o0 : o0 + np_s * 2 * OC])
                stripes.append(w)

            # One PSUM tile per output chunk: each matmul writes one bank,
            # and chunk c's staging depends only on chunk c's group.
            ps = [
                pp.tile([M, n], mybir.dt.float32, name=f"ps{c}")
                for c, (o0, n) in enumerate(OCHUNKS)
            ]
            import bisect

            def mm(j, c):
                o0, n = OCHUNKS[c]
                s = bisect.bisect_right(STRIPE_OFF, j) - 1
                jj = j - STRIPE_OFF[s]
                w4 = stripes[s][:, :].rearrange(
                    "p (jj i o) -> p jj i o", jj=STRIPE_PASSES[s], i=2, o=OC
                )
                nc.tensor.matmul(
                    ps[c][:, :],
                    x4[:, j],
                    w4[:, jj, :, o0 : o0 + n],
                    start=(j == 0),
                    stop=(j == NP - 1),
                    perf_mode=mybir.MatmulPerfMode.DoubleRow,
                )

            x4 = xs[:, :].rearrange("p (j i m) -> p j i m", j=NP, i=2, m=M)
            # j-major through pass 14: pass-14 matmuls run as soon as its
            # stripe lands (mid-stream) so only the 3 closing pass-15
            # matmuls remain after the final stripe's semaphore.
            for j in range(NP - 1):
                for c in range(len(OCHUNKS)):
                    mm(j, c)
                # filler matmuls: early j-groups are DMA-gated with PE-idle
                # gaps that keep resetting the HAM activity window; fillers
                # keep the PE busy until it reaches K=8/8 (2.4 GHz).
                if j < 4:
                    for f in range(2):
                        nc.tensor.matmul(
                            scratch[:, :], wsrc[:, 0:128], wsrc[:, 0:512],
                            start=True, stop=True,
                        )
            # close each chunk, then drain: c0 on vector and c1 on scalar
            # run fully parallel; c2 (smallest, closes last) splits its
            # CAST across both engines the moment they free up.
            dma_eng = (nc.sync, nc.scalar, nc.sync)
            sbs = []
            for c, (o0, n) in enumerate(OCHUNKS):
                mm(NP - 1, c)
                sb = op.tile([M, n], f16, name=f"sb{c}", tag=f"sb{c}")
                sbs.append(sb)
                if c == 0:
                    nc.vector.tensor_copy(sb[:, :], ps[c][:, :])
                elif c == 1:
                    nc.scalar.copy(sb[:, :], ps[c][:, :])
                else:
                    h = n // 2
                    nc.vector.tensor_copy(sb[:, 0:h], ps[c][:, 0:h])
                    nc.scalar.copy(sb[:, h:n], ps[c][:, h:n])
                dma_eng[c].dma_start(yp[:, o0 : o0 + n], sb[:, :])

    nc.finalize()
    return nc


def _get_nc():
    global _BUILT
    if _BUILT is None:
        _BUILT = _build()
    return _BUILT


def _fp8_split(v, nsplit):
    """Split v into fp8 planes: v ~= sum_q planes[q] / ALPHA**q."""
    planes = []
    rem = v.astype(np.float32)
    for q in range(nsplit):
        p = (rem * np.float32(ALPHA**q)).astype(ml_dtypes.float8_e4m3fn)
        planes.append(p)
        rem = rem - p.astype(np.float32) / np.float32(ALPHA**q)
    return planes


def _prep_inputs(x, weight, scale_factor):
    x = np.asarray(x, dtype=np.float32)
    weight = np.asarray(weight, dtype=np.float32)
    s = np.float32(np.asarray(scale_factor))

    xsT = (x * s).T.astype(np.float32)                  # [I, B]
    planes = _fp8_split(xsT, NSPLIT)
    stacked = np.concatenate(planes, axis=1)            # [I, M]
    # [I, M] with I = (j, i, p): k = 256j + 128i + p  ->  xt[p, j, i, m]
    xt = np.ascontiguousarray(
        stacked.reshape(NP, 2, 128, M).transpose(2, 0, 1, 3).reshape(128, NP * 2 * M)
    )

    in_maps = []
    for c in range(N_CORES):
        wc = weight[c * OC : (c + 1) * OC, :]           # [OC, I]
        wq = wc.T.astype(ml_dtypes.float8_e4m3fn)       # [I, OC], exact
        wtc = np.ascontiguousarray(
            wq.reshape(NP, 2, 128, OC).transpose(2, 0, 1, 3).reshape(128, NP * 2 * OC)
        )
        in_maps.append({"xt": xt, "wt": wtc})
    return in_maps


def _run(in_maps, trace=False, tmpdir=None):
    from concourse.bass_utils import run_bass_kernel_spmd

    return run_bass_kernel_spmd(
        _get_nc(), in_maps, core_ids=list(range(N_CORES)), trace=trace, tmpdir=tmpdir
    )


def _combine(yp):
    acc = yp[0:B].astype(np.float32).copy()
    for q in range(1, NSPLIT):
        acc += yp[q * B : (q + 1) * B].astype(np.float32) * np.float32(
            1.0 / ALPHA**q
        )
    return acc


def kernel(x, weight, scale_factor):
    in_maps = _prep_inputs(x, weight, scale_factor)
    try:
        res = _run(in_maps)
    except Exception:
        # transient runtime/device hiccups happen; one retry is cheap and
        # the output is still checked downstream
        res = _run(in_maps)
    return np.concatenate(
        [_combine(res.results[c]["yp"]) for c in range(N_CORES)], axis=1
    )



# revision 25
# speedup vs baseline: 1.1147x; 1.1008x over previous
"""BitNet ternary linear layer on 8 Trainium2 NeuronCores.

y = x @ (W * s)^T with x (32, 4096) f32, W (11008, 4096) ternary {-1,0,+1}.

Strategy (memory-bound — the kernel is a W-stream at HBM line rate):
  - Tensor-parallel: shard W rows (out_features) across 8 cores, 1376 each;
    x replicated; per-core [32, 1376] outputs concatenated on the host.
  - Host-side prep (free): fold s into x, transpose to PE layouts, store W
    as fp8 E4M3 (ternary is EXACT in fp8). x is split into NSPLIT=2 fp8
    planes stacked along the stationary M dim (~2^-8 x precision; measured
    rel err ~7e-4 vs the 2e-2 gate), so x DMA is only 262 KB.
  - fp8 DoubleRow matmuls: K=256 per pass, 16 passes accumulate into
    per-chunk PSUM tiles (one tile per 512-col output chunk so each chunk's
    PSUM->SBUF copy + out-DMA depends only on its own accumulation group
    and overlaps the remaining matmuls).
  - W DRAM layout is k-major per partition; stripes ring on both HWDGE
    queues (Sync + Scalar), small-first for bandwidth ramp, small-last so
    the final matmuls are gated on as few bytes as possible.
  - Last TAIL_PASSES passes run chunk-major: chunk c's accumulation closes
    early, staging/DMA for c overlaps matmuls of c+1.
  - Outputs leave as fp16 raw planes (cast during the PSUM->SBUF copy);
    the scaled plane-sum runs on the host in f32.
  - Warmup/filler matmuls keep the PE busy so the HAM clock gate reaches
    K=8/8 (2.4 GHz) early instead of idling back to 1.2 GHz.
"""

import numpy as np
import ml_dtypes

N_CORES = 8
B, I, O = 32, 4096, 11008
OC = O // N_CORES        # 1376
NP = I // 256            # 16 DoubleRow passes (K=256 each)
NSPLIT = 2               # fp8 planes of x
ALPHA = 16.0             # residual plane q scaled by ALPHA**q
M = NSPLIT * B           # 64 stationary columns
# W DMA stripes: (first_pass, n_passes, queue). Each HWDGE ring delivers
# its stripes FIFO at ~half the aggregate rate, and a stripe's
# consumer-visible arrival is its completion semaphore. Giving sync only
# 1-pass stripes and scalar 2-pass stripes makes the two rings' sem
# cadences incommensurate, so they cannot phase-align into multi-us
# arrival gaps (which idle the PE and re-throttle the HAM clock gate —
# a run-to-run lottery with symmetric stripe sizes). Last stripe is
# small and on the lighter ring so the closing matmuls gate on as few
# bytes as possible.
STRIPES = [
    (0, 1, "scalar"),
    (1, 1, "sync"),
    (2, 2, "scalar"),
    (4, 1, "sync"),
    (5, 1, "sync"),
    (6, 2, "scalar"),
    (8, 1, "sync"),
    (9, 1, "sync"),
    (10, 2, "scalar"),
    (12, 1, "sync"),
    (13, 1, "sync"),
    (14, 1, "scalar"),
    (15, 1, "sync"),
]
STRIPE_START = [p for p, _, _ in STRIPES]
OCHUNKS = [(0, 512), (512, 512), (1024, 352)]
WARMUP_MMS = 7

_BUILT = None


def _build():
    import concourse.bacc as bacc
    import concourse.mybir as mybir
    from concourse.tile import TileContext

    f8 = mybir.dt.float8e4
    f16 = mybir.dt.float16
    nc = bacc.Bacc("TRN2", target_bir_lowering=False, debug=False)
    xt = nc.dram_tensor("xt", (128, NP * 2 * M), f8, kind="ExternalInput")
    wt = nc.dram_tensor("wt", (128, NP * 2 * OC), f8, kind="ExternalInput")
    # raw per-plane partials in fp16; the scaled plane-sum happens on host
    yp = nc.dram_tensor("yp", (M, OC), f16, kind="ExternalOutput")

    with TileContext(nc) as tc:
        with (
            tc.tile_pool(name="xp", bufs=1) as xp,
            tc.tile_pool(name="wp", bufs=1) as wp,
            tc.tile_pool(name="pp", bufs=1, space="PSUM") as pp,
            tc.tile_pool(name="op", bufs=1) as op,
        ):
            # PE warmup: garbage matmuls on a memset tile (no DMA dependency)
            # into a scratch PSUM bank, warming HAM while x and W stripe 0
            # load.
            wsrc = xp.tile([128, 512], f8, name="wsrc")
            nc.gpsimd.memset(wsrc[:, :], 0.0)
            scratch = pp.tile([128, 512], mybir.dt.float32, name="scratch")
            for wu in range(WARMUP_MMS):
                nc.tensor.matmul(
                    scratch[:, :], wsrc[:, 0:128], wsrc[:, 0:512],
                    start=True, stop=True,
                )

            xs = xp.tile([128, NP * 2 * M], f8)
            nc.sync.dma_start(xs[:, :], xt[:, :])

            # stripe 0 rings on Scalar's HWDGE queue while the x DMA rings
            # on Sync's — both land together, real matmuls start early.
            stripes = []
            for s, (p0, np_s, q) in enumerate(STRIPES):
                w = wp.tile([128, np_s * 2 * OC], f8, name=f"w{s}", tag=f"w{s}")
                o0 = p0 * 2 * OC
                eng = nc.scalar if q == "scalar" else nc.sync
                eng.dma_start(w[:, :], wt[:, o0 : o0 + np_s * 2 * OC])
                stripes.append(w)

            # One PSUM tile per output chunk: each matmul writes one bank,
            # and chunk c's staging depends only on chunk c's group.
            ps = [
                pp.tile([M, n], mybir.dt.float32, name=f"ps{c}")
                for c, (o0, n) in enumerate(OCHUNKS)
            ]
            import bisect

            def mm(j, c):
                o0, n = OCHUNKS[c]
                s = bisect.bisect_right(STRIPE_START, j) - 1
                p0, np_s, _ = STRIPES[s]
                w4 = stripes[s][:, :].rearrange(
                    "p (jj i o) -> p jj i o", jj=np_s, i=2, o=OC
                )
                nc.tensor.matmul(
                    ps[c][:, :],
                    x4[:, j],
                    w4[:, j - p0, :, o0 : o0 + n],
                    start=(j == 0),
                    stop=(j == NP - 1),
                    perf_mode=mybir.MatmulPerfMode.DoubleRow,
                )

            x4 = xs[:, :].rearrange("p (j i m) -> p j i m", j=NP, i=2, m=M)
            # j-major through pass 14: pass-14 matmuls run as soon as its
            # stripe lands (mid-stream) so only the 3 closing pass-15
            # matmuls remain after the final stripe's semaphore.
            for j in range(NP - 1):
                for c in range(len(OCHUNKS)):
                    mm(j, c)
                # filler matmuls: early j-groups are DMA-gated with PE-idle
                # gaps that keep resetting the HAM activity window; fillers
                # keep the PE busy until it reaches K=8/8 (2.4 GHz).
                if j < 4:
                    for f in range(2):
                        nc.tensor.matmul(
                            scratch[:, :], wsrc[:, 0:128], wsrc[:, 0:512],
                            start=True, stop=True,
                        )
            # close each chunk, then drain: c0 on vector and c1 on scalar
            # run fully parallel; c2 (smallest, closes last) splits its
            # CAST across both engines the moment they free up.
            dma_eng = (nc.sync, nc.scalar, nc.sync)
            sbs = []
            for c, (o0, n) in enumerate(OCHUNKS):
                mm(NP - 1, c)
                sb = op.tile([M, n], f16, name=f"sb{c}", tag=f"sb{c}")
                sbs.append(sb)
                if c == 0:
                    nc.vector.tensor_copy(sb[:, :], ps[c][:, :])
                elif c == 1:
                    nc.scalar.copy(sb[:, :], ps[c][:, :])
                else:
                    h = n // 2
                    nc.vector.tensor_copy(sb[:, 0:h], ps[c][:, 0:h])
                    nc.scalar.copy(sb[:, h:n], ps[c][:, h:n])
                dma_eng[c].dma_start(yp[:, o0 : o0 + n], sb[:, :])

    nc.finalize()
    return nc


def _get_nc():
    global _BUILT
    if _BUILT is None:
        _BUILT = _build()
    return _BUILT


def _fp8_split(v, nsplit):
    """Split v into fp8 planes: v ~= sum_q planes[q] / ALPHA**q."""
    planes = []
    rem = v.astype(np.float32)
    for q in range(nsplit):
        p = (rem * np.float32(ALPHA**q)).astype(ml_dtypes.float8_e4m3fn)
        planes.append(p)
        rem = rem - p.astype(np.float32) / np.float32(ALPHA**q)
    return planes


def _prep_inputs(x, weight, scale_factor):
    x = np.asarray(x, dtype=np.float32)
    weight = np.asarray(weight, dtype=np.float32)
    s = np.float32(np.asarray(scale_factor))

    xsT = (x * s).T.astype(np.float32)                  # [I, B]
    planes = _fp8_split(xsT, NSPLIT)
    stacked = np.concatenate(planes, axis=1)            # [I, M]
    # [I, M] with I = (j, i, p): k = 256j + 128i + p  ->  xt[p, j, i, m]
    xt = np.ascontiguousarray(
        stacked.reshape(NP, 2, 128, M).transpose(2, 0, 1, 3).reshape(128, NP * 2 * M)
    )

    in_maps = []
    for c in range(N_CORES):
        wc = weight[c * OC : (c + 1) * OC, :]           # [OC, I]
        wq = wc.T.astype(ml_dtypes.float8_e4m3fn)       # [I, OC], exact
        wtc = np.ascontiguousarray(
            wq.reshape(NP, 2, 128, OC).transpose(2, 0, 1, 3).reshape(128, NP * 2 * OC)
        )
        in_maps.append({"xt": xt, "wt": wtc})
    return in_maps


def _run(in_maps, trace=False, tmpdir=None):
    from concourse.bass_utils import run_bass_kernel_spmd

    return run_bass_kernel_spmd(
        _get_nc(), in_maps, core_ids=list(range(N_CORES)), trace=trace, tmpdir=tmpdir
    )


def _combine(yp):
    acc = yp[0:B].astype(np.float32).copy()
    for q in range(1, NSPLIT):
        acc += yp[q * B : (q + 1) * B].astype(np.float32) * np.float32(
            1.0 / ALPHA**q
        )
    return acc


def kernel(x, weight, scale_factor):
    in_maps = _prep_inputs(x, weight, scale_factor)
    try:
        res = _run(in_maps)
    except Exception:
        # transient runtime/device hiccups happen; one retry is cheap and
        # the output is still checked downstream
        res = _run(in_maps)
    return np.concatenate(
        [_combine(res.results[c]["yp"]) for c in range(N_CORES)], axis=1
    )

